# revision 38
# baseline (speedup 1.0000x reference)
"""Trainium2 Bass kernel for nn_Attention_36481452212797.

Contract: kernel(**inputs) takes FULL inputs
  x [8, 4096, 256] f32, Wq/Wk/Wv [1024, 256], Wp [256, 1024], bp [256]
and returns the FULL output [8, 4096, 256] f32.

Sharding: data-parallel over B — one batch sample per NeuronCore, no
collectives. Per-core pipeline (per sample):

  xT = x.T                       (PE transposes)
  qT/q, kT/k = projections       (f32r matmuls, bf16 storage)
  DTA per stream (3-stage EM soft-clustering):
    bases0 = l2norm_c(maxpool32(qT))
    stage: zT = basesN.T @ qT    (bf16 MM, N=512)
           z  = softmax_KC(zT.T) (PE transpose + DVE/ACT)
           ybT = z.T @ q         (bf16 MM)
           basesT = l2norm_free(ybT)
    (the reference's l2norm of z over N cancels into the bases l2norm up
     to O(1e-8) — skipped)
  att_h = softmax_e(qbT_h.T @ kbT_h * SCALE)     (f32r)
  o_h   = attT_h.T @ vT_h                        (f32r, fused with final)
  out   = relu(o.T @ WpT + bp)                   (f32r, bias via K=1 matmul)

float32r is the PE's fast fp32 path (1 cycle/row at N>=256, ~1e-3 rel err);
bf16 is used only inside the DTA streams where the EM averaging washes the
rounding noise out (numpy-validated: end-to-end maxabs/scale ~3e-4).
"""

import copy
import sys
from contextlib import ExitStack

import numpy as np

sys.path.insert(0, "/opt/trn_rl_repo")

import concourse.bass as bass
import concourse.mybir as mybir
import concourse.tile as tile
from concourse.bass_utils import run_bass_kernel_spmd
from concourse.masks import make_identity

B, N, C, H, KC, STAGES = 8, 4096, 256, 8, 128, 3
C4 = 4 * C          # 1024
HD = C4 // H        # 128
SCALE = (C // H) ** -0.5
NT = N // 128       # 32 token tiles
NCH = C4 // 128     # 8 channel chunks
CCH = C // 128      # 2 input-channel chunks
W = N // KC         # 32: maxpool window

F32 = mybir.dt.float32
F32R = mybir.dt.float32r
BF16 = mybir.dt.bfloat16
AX = mybir.AxisListType
ALU = mybir.AluOpType
ACT = mybir.ActivationFunctionType


def cap_waits(nc, nop_templates, max_waits=1):
    """The walrus build here rejects instructions carrying more than one
    sync-wait command. Move excess waits onto EVSEM no-op carriers inserted
    before the capped instruction on the same engine."""
    m = nc.m
    new_m = copy.replace(m, functions=[])
    n_carriers = 0
    for function in m.functions:
        new_f = copy.replace(function, blocks=[])
        new_f.set_allocations_from_list(function.allocations)
        for block in function.blocks:
            new_insts = []
            for inst in block.instructions:
                si = inst.sync_info
                if si is not None and si.on_wait and len(si.on_wait) > max_waits:
                    waits = list(si.on_wait)
                    for w in waits[: len(waits) - max_waits]:
                        nop = copy.replace(
                            nop_templates[inst.engine],
                            name=f"{inst.name}-wc{n_carriers}",
                        )
                        tsi = nop_templates[inst.engine].sync_info
                        nop.sync_info = mybir.SyncInfo(
                            on_wait=[w],
                            on_update=list(tsi.on_update) if tsi else [],
                        )
                        new_insts.append(nop)
                        n_carriers += 1
                    inst.sync_info = mybir.SyncInfo(
                        on_wait=waits[len(waits) - max_waits :],
                        on_update=list(si.on_update or []),
                    )
                new_insts.append(inst)
            new_block = copy.replace(block, instructions=new_insts)
            new_f.blocks.append(new_block)
        new_m.functions.append(new_f)
    nc.m = new_m
    return n_carriers


def build_module():
    nc = bass.Bass()
    _dummy = nc.alloc_semaphore("waitcap_dummy")
    nop_templates = {
        e.ins.engine: e.ins
        for e in (
            nc.tensor.sem_inc(_dummy, 0),
            nc.vector.sem_inc(_dummy, 0),
            nc.scalar.sem_inc(_dummy, 0),
            nc.gpsimd.sem_inc(_dummy, 0),
            nc.sync.sem_inc(_dummy, 0),
        )
    }

    x_d = nc.declare_dram_parameter("x", [N, C], F32, isOutput=False)
    w_d = {
        "q": nc.declare_dram_parameter("Wq", [C4, C], F32, isOutput=False),
        "k": nc.declare_dram_parameter("Wk", [C4, C], F32, isOutput=False),
        "v": nc.declare_dram_parameter("Wv", [C4, C], F32, isOutput=False),
    }
    wp_d = nc.declare_dram_parameter("Wp", [C, C4], F32, isOutput=False)
    bp_d = nc.declare_dram_parameter("bp", [1, C], F32, isOutput=False)
    out_d = nc.declare_dram_parameter("out", [N, C], F32, isOutput=True)
    xT_dram = nc.dram_tensor("xT_scratch", [128, CCH * N], F32)

    with tile.TileContext(nc) as tc, ExitStack() as ctx:
        consts = ctx.enter_context(tc.tile_pool(name="consts", bufs=1))
        # PSUM: 3 + 3 + 2 banks = 8
        ps_mm = ctx.enter_context(tc.tile_pool(name="ps_mm", bufs=3, space="PSUM"))
        ps_tr = ctx.enter_context(tc.tile_pool(name="ps_tr", bufs=3, space="PSUM"))
        ps_sm = ctx.enter_context(tc.tile_pool(name="ps_sm", bufs=2, space="PSUM"))
        work = ctx.enter_context(tc.tile_pool(name="work", bufs=2))

        ident = consts.tile([128, 128], F32)
        make_identity(nc, ident[:])
        identr = consts.tile([128, 128], F32R)
        nc.vector.tensor_copy(identr[:], ident[:])

        ones_f = consts.tile([1, 128], F32)
        nc.vector.memset(ones_f[:], 1.0)
        ones_r = consts.tile([1, 128], F32R)
        nc.vector.tensor_copy(ones_r[:], ones_f[:])
        bp_f = consts.tile([1, C], F32)
        nc.sync.dma_start(bp_f[:], bp_d[:])
        bp_r = consts.tile([1, C], F32R)
        nc.vector.tensor_copy(bp_r[:], bp_f[:])

        qbT = consts.tile([128, C4], F32R, tag="qbT")
        kbT = consts.tile([128, C4], F32R, tag="kbT")

        def psum_copy(dst_ap, src_ap, idx, act_heavy=False):
            """Copy PSUM->SBUF alternating DVE/ACT to balance engine load.
            act_heavy routes 2/3 to ACT (projection phases keep DVE busy
            with reduces)."""
            dve = (idx % 6 == 0) if act_heavy else (idx % 2 == 0)
            if dve:
                nc.vector.tensor_copy(dst_ap, src_ap)
            else:
                nc.scalar.copy(dst_ap, src_ap)

        _tr_idx = [0]

        def pe_transpose(src_ap, dst_ap):
            """dst = src.T for one [128,128] fp32 block via PE."""
            ps = ps_tr.tile([128, 128], F32, tag="tr")
            nc.tensor.transpose(ps[:], src_ap, ident[:])
            _tr_idx[0] += 1
            psum_copy(dst_ap, ps[:], _tr_idx[0])

        def softmax_free(src_psum, out_ap, p, f, scale=1.0):
            """out = softmax over free axis of src_psum [p, f]. The inputs
            here are bounded (|logit| <= ~12), so the max-subtraction is
            skipped — exp stays comfortably inside fp32 range."""
            ex = work.tile([p, f], F32, tag="sm_exp", bufs=4)
            ssum = work.tile([p, 1], F32, tag="sm_sum", bufs=4)
            nc.scalar.activation(
                out=ex[:], in_=src_psum, func=ACT.Exp,
                scale=float(scale), accum_out=ssum[:],
            )
            rec = work.tile([p, 1], F32, tag="sm_rec", bufs=4)
            nc.vector.reciprocal(rec[:], ssum[:])
            nc.vector.tensor_scalar_mul(out_ap, ex[:], rec[:])

        def l2norm_free(src_ap, dst_ap, p, f):
            """dst = src / (1e-6 + l2norm of src row) over the free axis.
            sum(x^2) = f*(var + mean^2) via bn_stats (no big scratch)."""
            nsub = (f + 511) // 512
            sub = f // nsub
            src3 = src_ap.rearrange("p (n s) -> p n s", s=sub)
            stats = work.tile([p, nsub, 6], F32, tag="l2_stats")
            for i in range(nsub):
                nc.vector.bn_stats(out=stats[:, i, :], in_=src3[:, i, :])
            mv = work.tile([p, 2], F32, tag="l2_mv")
            nc.vector.bn_aggr(out=mv[:], in_=stats[:])
            m2 = work.tile([p, 1], F32, tag="l2_m2")
            nc.vector.tensor_mul(m2[:], mv[:, 0:1], mv[:, 0:1])
            nc.vector.tensor_add(m2[:], m2[:], mv[:, 1:2])
            nrm = work.tile([p, 1], F32, tag="l2_nrm")
            nc.scalar.activation(
                out=nrm[:], in_=m2[:], func=ACT.Sqrt, scale=float(f)
            )
            nc.vector.tensor_scalar_add(nrm[:], nrm[:], 1e-6)
            rec = work.tile([p, 1], F32, tag="l2_rec")
            nc.vector.reciprocal(rec[:], nrm[:])
            nc.vector.tensor_scalar_mul(dst_ap, src_ap, rec[:])

        def load_xT(pool, first):
            """First call: load x, transpose into xT [128, CCH, N] f32r and
            spill to DRAM. Later calls: reload the spilled copy."""
            xT = pool.tile([128, CCH, N], F32R, tag="xT")
            xT_flat = xT[:].rearrange("p a b -> p (a b)").bitcast(F32)
            Q = CCH * N // 4
            if first:
                for t4 in range(NT // 4):
                    xtile = work.tile([128, 4, C], F32, tag="ld")
                    eng = nc.sync if t4 % 2 == 0 else nc.gpsimd
                    eng.dma_start(
                        xtile[:],
                        x_d[bass.ds(t4 * 512, 512), :].rearrange(
                            "(a p) c -> p a c", p=128
                        ),
                    )
                    for a in range(4):
                        t = t4 * 4 + a
                        for j in range(CCH):
                            pe_transpose(
                                xtile[:, a, bass.ts(j, 128)],
                                xT[:, j, bass.ts(t, 128)],
                            )
                for i in range(4):
                    eng = nc.sync if i % 2 == 0 else nc.gpsimd
                    eng.dma_start(
                        xT_dram[:, bass.ds(i * Q, Q)], xT_flat[:, bass.ds(i * Q, Q)]
                    )
            else:
                for i in range(4):
                    eng = nc.sync if i % 2 == 0 else nc.gpsimd
                    eng.dma_start(
                        xT_flat[:, bass.ds(i * Q, Q)], xT_dram[:, bass.ds(i * Q, Q)]
                    )
            return xT

        def load_wT(pool, wd, dt=F32R):
            """Load one q/k/v weight and transpose into [128, CCH, C4]."""
            wT = pool.tile([128, CCH, C4], dt, tag="wT")
            for i2 in range(2):
                wtile = work.tile([128, 4, C], F32, tag="ld")
                eng = nc.sync if i2 % 2 == 0 else nc.gpsimd
                eng.dma_start(
                    wtile[:],
                    wd[bass.ds(i2 * 512, 512), :].rearrange("(a p) c -> p a c", p=128),
                )
                for a in range(4):
                    i = i2 * 4 + a
                    for j in range(CCH):
                        pe_transpose(
                            wtile[:, a, bass.ts(j, 128)], wT[:, j, bass.ts(i, 128)]
                        )
            return wT

        def projection_T(wT, xT_ap, dst_big, maxpool_to=None, t8s=None):
            """dst[c4, n] = W @ x.T as psum tiles [128, 512]. When
            maxpool_to is given, also reduce each psum tile over 32-token
            windows into it (bases0 seed, fused to overlap with the MMs)."""
            for i in range(NCH):
                for t8 in t8s if t8s is not None else range(N // 512):
                    ps = ps_mm.tile([128, 512], F32, tag="mm")
                    for j in range(CCH):
                        nc.tensor.matmul(
                            ps[:],
                            wT[:, j, bass.ts(i, 128)],
                            xT_ap(j, t8),
                            start=(j == 0),
                            stop=(j == CCH - 1),
                        )
                    psum_copy(
                        dst_big[:, i, bass.ds(t8 * 512, 512)], ps[:],
                        i + t8, act_heavy=True,
                    )
                    if maxpool_to is not None and t8 == (N // 512) - 1:
                        nc.vector.tensor_reduce(
                            maxpool_to[:, i, :],
                            dst_big[:, i, :].rearrange("p (k w) -> p k w", w=W),
                            axis=AX.X,
                            op=ALU.max,
                        )

        def projection_nat(wT, xT, dst_big):
            """dst[n, c4] = x @ W.T ; lhsT = xT tiles, rhs = WT chunks."""
            for t in range(NT):
                for c8 in range(C4 // 512):
                    ps = ps_mm.tile([128, 512], F32, tag="mm")
                    for j in range(CCH):
                        nc.tensor.matmul(
                            ps[:],
                            xT[:, j, bass.ts(t, 128)],
                            wT[:, j, bass.ds(c8 * 512, 512)],
                            start=(j == 0),
                            stop=(j == CCH - 1),
                        )
                    psum_copy(dst_big[:, t, bass.ds(c8 * 512, 512)], ps[:], t + c8, act_heavy=True)

        def dta_branch(stage_pool, sT_big, s_big, mx_big, out_basesT):
            """EM clustering on one stream; writes normalized bases (basesT
            layout [KC, C4]) into out_basesT (f32r). mx_big holds the fused
            maxpool seed from projection_T."""
            basesT = stage_pool.tile([128, C4], F32, tag="basesT")
            basesN = stage_pool.tile([128, NCH, 128], BF16, tag="basesN")
            z_big = stage_pool.tile([128, NT, KC], BF16, tag="z")

            for i in range(NCH):
                pe_transpose(mx_big[:, i, :], basesT[:, bass.ts(i, 128)])
            l2norm_free(basesT[:], basesT[:], 128, C4)

            for s in range(STAGES):
                # basesN <- basesT.T (bf16) for the stage-A matmul
                for i in range(NCH):
                    pe_transpose(basesT[:, bass.ts(i, 128)], basesN[:, i, :])

                # stage A: zT[k, n] = sum_c basesN[c,k] * sT[c,n];
                # then per 128-token block: PE transpose + softmax over KC
                for t8 in range(N // 512):
                    ps = ps_mm.tile([128, 512], F32, tag="mm")
                    for i in range(NCH):
                        nc.tensor.matmul(
                            ps[:],
                            basesN[:, i, :],
                            sT_big[:, i, bass.ds(t8 * 512, 512)],
                            start=(i == 0),
                            stop=(i == NCH - 1),
                        )
                    zst = work.tile([128, 512], F32R, tag="zstage")
                    nc.vector.tensor_copy(zst[:], ps[:])
                    for tt in range(4):
                        psz = ps_tr.tile([128, 128], F32R, tag="tr")
                        nc.tensor.matmul(
                            psz[:], zst[:, bass.ts(tt, 128)], identr[:],
                            is_transpose=True, start=True, stop=True,
                        )
                        softmax_free(psz[:], z_big[:, t8 * 4 + tt, :], 128, KC)

                # stage B: ybT[k, c] = sum_n z[n,k] * s[n,c]
                for c2 in range(C4 // 512):
                    ps = ps_mm.tile([128, 512], F32, tag="mm")
                    for t in range(NT):
                        nc.tensor.matmul(
                            ps[:],
                            z_big[:, t, :],
                            s_big[:, t, bass.ds(c2 * 512, 512)],
                            start=(t == 0),
                            stop=(t == NT - 1),
                        )
                    nc.vector.tensor_copy(
                        basesT[:, bass.ds(c2 * 512, 512)], ps[:]
                    )
                if s < STAGES - 1:
                    l2norm_free(basesT[:], basesT[:], 128, C4)
            l2norm_free(basesT[:], out_basesT, 128, C4)

        # ---- q and k branches (sequential; they share the big buffers) ----
        with ExitStack() as br_ctx:
            streams = br_ctx.enter_context(tc.tile_pool(name="streams", bufs=1))
            sT_big = streams.tile([128, NCH, N], BF16, tag="sT")
            s_big = streams.tile([128, NT, C4], BF16, tag="s_nat")
            mx_big = streams.tile([128, NCH, KC], F32, tag="mx")

            # q branch: f32r projection, builds + spills xT
            with ExitStack() as proj_ctx:
                ppool = proj_ctx.enter_context(tc.tile_pool(name="proj_q", bufs=1))
                wT = load_wT(ppool, w_d["q"])
                xT = load_xT(ppool, first=True)
                projection_T(
                    wT,
                    lambda j, t8: xT[:, j, bass.ds(t8 * 512, 512)],
                    sT_big,
                    maxpool_to=mx_big,
                )
                projection_nat(wT, xT, s_big)
            # bf16 copy of xT for the k projection, via casting SWDGE DMA
            # (runs during q's DTA while the DMA engines are idle; k only
            # feeds the error-tolerant EM clustering, bf16 is enough)
            xbf_pool = br_ctx.enter_context(tc.tile_pool(name="xbf", bufs=1))
            xTbf = xbf_pool.tile([128, CCH, N], BF16, tag="xTbf")
            xTbf_flat = xTbf[:].rearrange("p a b -> p (a b)")
            Q4 = CCH * N // 4
            for i in range(4):
                nc.gpsimd.dma_start(
                    xTbf_flat[:, bass.ds(i * Q4, Q4)],
                    xT_dram[:, bass.ds(i * Q4, Q4)],
                )
            with ExitStack() as st_ctx:
                stage_pool = st_ctx.enter_context(
                    tc.tile_pool(name="stage_q", bufs=1)
                )
                dta_branch(stage_pool, sT_big, s_big, mx_big, qbT[:])

            # k branch: all-bf16 projection from the resident xTbf
            with ExitStack() as proj_ctx:
                ppool = proj_ctx.enter_context(tc.tile_pool(name="proj_k", bufs=1))
                wTk = load_wT(ppool, w_d["k"], dt=BF16)
                projection_T(
                    wTk,
                    lambda j, t8: xTbf[:, j, bass.ds(t8 * 512, 512)],
                    sT_big,
                    maxpool_to=mx_big,
                )
                projection_nat(wTk, xTbf, s_big)
            with ExitStack() as st_ctx:
                stage_pool = st_ctx.enter_context(
                    tc.tile_pool(name="stage_k", bufs=1)
                )
                dta_branch(stage_pool, sT_big, s_big, mx_big, kbT[:])

        # ---- v projection, attention, output projection ----
        with ExitStack() as v_ctx:
            vpool = v_ctx.enter_context(tc.tile_pool(name="vpool", bufs=1))
            vT = vpool.tile([128, NCH, N], F32R, tag="vT")
            with ExitStack() as proj_ctx:
                ppool = proj_ctx.enter_context(tc.tile_pool(name="proj_v", bufs=1))
                wT = load_wT(ppool, w_d["v"])
                NH = N // 2
                for half in range(2):
                    xTh = ppool.tile([128, CCH, NH], F32R, tag="xTh")
                    xTh_flat = xTh[:].rearrange("p a b -> p (a b)").bitcast(F32)
                    for j in range(CCH):
                        eng = nc.sync if j % 2 == 0 else nc.gpsimd
                        eng.dma_start(
                            xTh_flat[:, bass.ds(j * NH, NH)],
                            xT_dram[:, bass.ds(j * N + half * NH, NH)],
                        )
                    projection_T(
                        wT,
                        lambda j, t8: xTh[:, j, bass.ds(t8 * 512 - half * NH, 512)],
                        vT,
                        t8s=range(half * 4, (half + 1) * 4),
                    )

            # WpT [128, NCH, C] f32r
            wpT = vpool.tile([128, NCH, C], F32R, tag="wpT")
            for i in range(CCH):
                for jj in range(4):
                    wtile = work.tile([128, C], F32, tag="ld")
                    nc.sync.dma_start(
                        wtile[:], wp_d[bass.ts(i, 128), bass.ds(jj * 256, 256)]
                    )
                    for j2 in range(2):
                        j = jj * 2 + j2
                        pe_transpose(
                            wtile[:, bass.ts(j2, 128)],
                            wpT[:, j, bass.ts(i, 128)],
                        )

            # attention per head: att = softmax_e(qh . kh^T * SCALE), then
            # transpose (f32r) for the o-matmul
            attT = vpool.tile([128, H, 128], F32R, tag="attT")
            att_s = vpool.tile([128, H, 128], F32R, tag="att_s")
            for h in range(H):
                psa = ps_sm.tile([128, 128], F32, tag="sm")
                nc.tensor.matmul(
                    psa[:],
                    qbT[:, bass.ts(h, 128)],
                    kbT[:, bass.ts(h, 128)],
                    start=True,
                    stop=True,
                )
                softmax_free(psa[:], att_s[:, h, :], 128, 128, scale=SCALE)
                pst = ps_tr.tile([128, 128], F32R, tag="tr")
                nc.tensor.matmul(
                    pst[:], att_s[:, h, :], identr[:],
                    is_transpose=True, start=True, stop=True,
                )
                nc.vector.tensor_copy(attT[:, h, :], pst[:])

            # o = attT.T @ vT, fused per 512-token chunk with the output
            # projection (+ bias via K=1 matmul) and relu
            oc_pool = v_ctx.enter_context(tc.tile_pool(name="oc", bufs=1))
            for t8 in range(N // 512):
                oc = oc_pool.tile([128, H, 512], F32R, tag="oc")
                for h in range(H):
                    ps = ps_mm.tile([128, 512], F32, tag="mm")
                    nc.tensor.matmul(
                        ps[:],
                        attT[:, h, :],
                        vT[:, h, bass.ds(t8 * 512, 512)],
                        start=True,
                        stop=True,
                    )
                    psum_copy(oc[:, h, :], ps[:], h)
                obig = work.tile([128, 4, C], F32, tag="obig")
                for tt in range(4):
                    pso = ps_sm.tile([128, C], F32, tag="sm")
                    for h in range(H):
                        nc.tensor.matmul(
                            pso[:],
                            oc[:, h, bass.ts(tt, 128)],
                            wpT[:, h, :],
                            start=(h == 0),
                            stop=False,
                        )
                    nc.tensor.matmul(
                        pso[:], ones_r[:], bp_r[:], start=False, stop=True
                    )
                    nc.scalar.activation(
                        out=obig[:, tt, :], in_=pso[:], func=ACT.Relu
                    )
                eng = nc.sync if t8 % 2 == 0 else nc.gpsimd
                eng.dma_start(
                    out_d[bass.ds(t8 * 512, 512), :].rearrange(
                        "(a p) c -> p a c", p=128
                    ),
                    obig[:],
                )

    cap_waits(nc, nop_templates)
    return nc


_NC_CACHE = None


def _get_module():
    global _NC_CACHE
    if _NC_CACHE is None:
        _NC_CACHE = build_module()
    return _NC_CACHE


def _in_maps(inputs):
    x = np.ascontiguousarray(inputs["x"], dtype=np.float32)
    shared = {
        "Wq": np.ascontiguousarray(inputs["Wq"], dtype=np.float32),
        "Wk": np.ascontiguousarray(inputs["Wk"], dtype=np.float32),
        "Wv": np.ascontiguousarray(inputs["Wv"], dtype=np.float32),
        "Wp": np.ascontiguousarray(inputs["Wp"], dtype=np.float32),
        "bp": np.ascontiguousarray(inputs["bp"], dtype=np.float32).reshape(1, C),
    }
    return [{"x": x[b], **shared} for b in range(B)]


def kernel(**inputs) -> np.ndarray:
    nc = _get_module()
    res = run_bass_kernel_spmd(nc, _in_maps(inputs), core_ids=list(range(B)))
    return np.stack([res.results[b]["out"] for b in range(B)], axis=0)


def run_traced(**inputs):
    """kernel() with NTFF tracing; returns (output, BassKernelResults)."""
    nc = _get_module()
    res = run_bass_kernel_spmd(
        nc, _in_maps(inputs), core_ids=list(range(B)), trace=True
    )
    out = np.stack([res.results[b]["out"] for b in range(B)], axis=0)
    return out, res
